# revision 34
# baseline (speedup 1.0000x reference)
"""Trainium2 Bass kernel for DangoPreTrain-style GNN pretraining forward.

Model (per edge type t of 3):
    h1 = relu(SAGE(emb, edges_t; W1l, b1, W1r))
    h2 = relu(SAGE(h1,  edges_t; W2l, b2, W2r))
    recon = h2 @ Wrec_t.T + brec_t            # [N, N]
Returns (embeddings [3,N,64], reconstructions [3,N,N], x_init [N,64]).

Strategy: the sparse mean-aggregation agg[i] = mean_{(j->i)} x[j] is
reformulated as a dense GEMM with the transposed edge-count matrix
CT[src, dst] = #edges(src->dst) (counts <= ~4, exact in fp8_e4m3):
aggT = x^T @ CT, then scale columns by 1/deg. CT is built on host
(bincount), chunk-packed in fp8_e4m3, and sharded over the 8 NeuronCores
by destination-node blocks (832 columns each). One SPMD launch does
everything, software-pipelined one edge type deep: each core loads its
CT shard once into SBUF and reuses it for both SAGE layers; h1 is
exchanged between cores with an on-device AllGather (layer-1 output is
PE-transposed to row layout first); the AllGather of type t overlaps
layer-1 of type t+1 and the recon slab of type t-1. Aggregation GEMMs
run bf16 x fp8 with fp32 PSUM accumulation; the 64x64 linear transforms
run in float32r; recon drains PSUM->SBUF on DVE+ACT and streams
contiguous row-stripes to HBM.
"""

import os
import sys

import numpy as np
import ml_dtypes

sys.path.insert(0, "/opt/trn_rl_repo")

import concourse.bacc as bacc
import concourse.mybir as mybir
import concourse.tile as tile
from concourse.bass_utils import run_bass_kernel_spmd


def _ensure_ntff_hook():
    """Register the NTFF profile hook missing from this image's antenv.

    trn_boot tries to install it but antenv lacks the axon_hooks module;
    shim it in-process so run_bass_kernel_spmd(trace=True) can profile.
    """
    try:
        from antenv import axon_hooks  # noqa: F401
        return True
    except ImportError:
        pass
    try:
        import types
        import antenv
        if "/root/.axon_site" not in sys.path:
            sys.path.insert(0, "/root/.axon_site")
        from trn_agent_boot.trn_boot import _ntff_profile_via_ctypes
        hook = _ntff_profile_via_ctypes("/opt/axon/libaxon_pjrt.so")
        if hook is None:
            return False
        m = types.ModuleType("antenv.axon_hooks")
        m.get_axon_ntff_profile_hook = lambda: hook
        m.set_axon_ntff_profile_hook = lambda h: None
        sys.modules["antenv.axon_hooks"] = m
        antenv.axon_hooks = m
        return True
    except Exception:
        return False


BF16 = ml_dtypes.bfloat16
F32 = mybir.dt.float32
BF = mybir.dt.bfloat16
RELU = mybir.ActivationFunctionType.Relu

N = 6607      # nodes
H = 64        # hidden
T = 3         # edge types
NP = 6656     # N padded to 52*128
KCH = NP // 128   # 52 contraction chunks
NC = 8        # cores
S = NP // NC  # 832 dst columns per core
FB = [(0, 512), (512, 320)]          # free-dim blocks of the 832-wide shard
IB = [(i * 128, 128) for i in range(6)] + [(768, 64)]  # 832-row blocks
NB = [(i * 512, 512) for i in range(13)]               # recon col blocks

LAST_EXEC_NS = None
LAST_NS_PARTS = None


def _build_merged():
    """Single-launch kernel, software-pipelined one type deep:
    L1(t) -> AllGather(t) overlaps L1(t+1); L2(t)/recon(t) run after.
    CT shard loaded once per type (fp8_e4m3, counts exact), reused by
    both layers from SBUF."""
    from concourse.masks import make_identity
    FP8 = mybir.dt.float8e4
    F32R = mybir.dt.float32r

    nc = bacc.Bacc("TRN2", target_bir_lowering=False, debug=False,
                   num_devices=NC)
    ctc = nc.dram_tensor("ctc", [T, 128, KCH * S], FP8, kind="ExternalInput")
    xch = nc.dram_tensor("xch", [128, KCH * H], BF, kind="ExternalInput")
    xT = nc.dram_tensor("xT", [H, S], F32R, kind="ExternalInput")
    rb = nc.dram_tensor("rb", [T, H, S], F32, kind="ExternalInput")
    w1l = nc.dram_tensor("w1l", [T, H, H], F32R, kind="ExternalInput")
    w1r = nc.dram_tensor("w1r", [T, H, H], F32R, kind="ExternalInput")
    b1 = nc.dram_tensor("b1", [T, H, 1], F32, kind="ExternalInput")
    w2l = nc.dram_tensor("w2l", [T, H, H], F32R, kind="ExternalInput")
    w2r = nc.dram_tensor("w2r", [T, H, H], F32R, kind="ExternalInput")
    b2 = nc.dram_tensor("b2", [T, H, 1], F32, kind="ExternalInput")
    wrp = nc.dram_tensor("wrp", [T, H + 1, NP], BF, kind="ExternalInput")
    h2T = nc.dram_tensor("h2T", [T, H, S], F32, kind="ExternalOutput")
    recon = nc.dram_tensor("recon", [T, S, NP], F32, kind="ExternalOutput")

    CA = 7 * 512
    CB = NP - CA

    with tile.TileContext(nc) as tc:
        with (
            tc.tile_pool(name="const", bufs=1) as const,
            tc.tile_pool(name="ctp", bufs=2) as ctp,
            tc.tile_pool(name="hin", bufs=1) as hin,
            tc.tile_pool(name="wrpp", bufs=2) as wrpp,
            tc.tile_pool(name="wt", bufs=2) as wt,
            tc.tile_pool(name="work", bufs=2) as work,
            tc.tile_pool(name="outp", bufs=2) as outp,
            tc.tile_pool(name="psum_a", bufs=1, space="PSUM") as psum_a,
            tc.tile_pool(name="psum_w", bufs=1, space="PSUM") as psum_w,
            tc.tile_pool(name="psum_t", bufs=1, space="PSUM") as psum_t,
            tc.tile_pool(name="psum_r", bufs=4, space="PSUM") as psum_r,
            tc.tile_pool(name="dram", bufs=3, space="DRAM") as dram,
        ):
            xch_tile = const.tile([128, KCH * H], BF)
            nc.sync.dma_start(xch_tile[:], xch[:])
            xT_tile = const.tile([H, S], F32R)
            nc.sync.dma_start(xT_tile[:], xT[:])
            ident = const.tile([H, H], BF)
            make_identity(nc, ident)

            def loads(t):
                ct_big = ctp.tile([128, KCH * S], FP8, tag="ct",
                                  name="ct_big")
                nc.sync.dma_start(ct_big[:], ctc[t])
                wrp_tile = wrpp.tile([H + 1, NP], BF, tag="wrp",
                                     name="wrp_tile")
                nc.sync.dma_start(wrp_tile[:], wrp[t])
                tl = dict(ct=ct_big, wrp=wrp_tile)
                for nm, src_ in (("rb", rb), ("w1l", w1l), ("w1r", w1r),
                                 ("w2l", w2l), ("w2r", w2r)):
                    dt_ = F32 if nm == "rb" else F32R
                    shp = [H, S] if nm == "rb" else [H, H]
                    tile_ = wt.tile(shp, dt_, tag=nm, name=nm + "_t")
                    nc.sync.dma_start(tile_[:], src_[t])
                    tl[nm] = tile_
                for nm, src_ in (("b1", b1), ("b2", b2)):
                    tile_ = wt.tile([H, 1], F32, tag=nm, name=nm + "_t")
                    nc.sync.dma_start(tile_[:], src_[t])
                    tl[nm] = tile_
                return tl

            def agg(lhs_tile, ct_big, rb_tile, wl, wr, xTr, bcol, out):
                psA = psum_a.tile([64, 512], F32, tag="psA", name="psA")
                psB = psum_a.tile([64, 320], F32, tag="psB", name="psB")
                for k in range(KCH):
                    lhs = lhs_tile[:, k * H:(k + 1) * H]
                    nc.tensor.matmul(psA[:], lhs,
                                     ct_big[:, k * S:k * S + 512],
                                     start=(k == 0), stop=(k == KCH - 1))
                    nc.tensor.matmul(psB[:], lhs,
                                     ct_big[:, k * S + 512:(k + 1) * S],
                                     start=(k == 0), stop=(k == KCH - 1))
                mm = work.tile([64, S], mybir.dt.float32r, tag="mm",
                               name="mm", bufs=1)
                nc.vector.tensor_mul(mm[:, 0:512], psA[:], rb_tile[:, 0:512])
                nc.vector.tensor_mul(mm[:, 512:S], psB[:], rb_tile[:, 512:S])
                for (f0, fs) in FB:
                    psW = psum_w.tile([64, 512], F32, name="psW")
                    nc.tensor.matmul(psW[:, :fs], wl[:], mm[:, f0:f0 + fs],
                                     start=True, stop=False)
                    nc.tensor.matmul(psW[:, :fs], wr[:], xTr[:, f0:f0 + fs],
                                     start=False, stop=True)
                    nc.scalar.activation(out[:, f0:f0 + fs], psW[:, :fs],
                                         RELU, bias=bcol[:])

            def do_l1(t, tl):
                h1T_own = work.tile([H, S], mybir.dt.float32r,
                                    tag="h1T_own", name="h1T_own")
                agg(xch_tile, tl["ct"], tl["rb"], tl["w1l"], tl["w1r"],
                    xT_tile, tl["b1"], h1T_own)
                h1Tb = work.tile([H, S], BF, tag="h1Tb", name="h1Tb",
                                 bufs=1)
                nc.vector.tensor_copy(h1Tb[:], h1T_own[:])
                gin = dram.tile([S, H], BF, tag="gin", name="gin")
                gout = dram.tile([NP, H], BF, tag="gout", name="gout",
                                 addr_space="Shared")
                for (j0, jw) in IB:
                    psT = psum_t.tile([128, H], BF, name="psT")
                    nc.tensor.transpose(psT[:jw, :], h1Tb[:, j0:j0 + jw],
                                        ident[:])
                    trs = work.tile([128, H], BF, tag="trs", name="trs")
                    nc.vector.tensor_copy(trs[:jw, :], psT[:jw, :])
                    nc.scalar.dma_start(gin[j0:j0 + jw, :], trs[:jw, :])
                nc.gpsimd.collective_compute(
                    "AllGather", mybir.AluOpType.bypass,
                    replica_groups=[list(range(NC))],
                    ins=[gin[:]], outs=[gout[:]],
                )
                return h1T_own, gout

            def do_l2(t, tl, h1T_own, gout):
                h1ch = hin.tile([128, KCH * H], BF, name="h1ch")
                for k in range(KCH):
                    nc.scalar.dma_start(h1ch[:, k * H:(k + 1) * H],
                                        gout[k * 128:(k + 1) * 128, :])
                hf = outp.tile([H, S], F32, tag="hf", name="hf", bufs=1)
                agg(h1ch, tl["ct"], tl["rb"], tl["w2l"], tl["w2r"],
                    h1T_own, tl["b2"], hf)
                nc.scalar.dma_start(h2T[t], hf[:])
                h2b = work.tile([H + 1, S], BF, tag="h2b", name="h2b",
                                bufs=1)
                nc.vector.tensor_copy(h2b[0:H, :], hf[:])
                nc.vector.memset(h2b[H:H + 1, :], 1.0)
                return h2b

            def emit_recon(t, h2b, wrp_tile):
                for (i0, isz) in IB:
                    stA = outp.tile([128, CA], F32, tag="stA", name="stA")
                    stB = outp.tile([128, CB], F32, tag="stB", name="stB")
                    for j, (n0, nsz) in enumerate(NB):
                        psR = psum_r.tile([128, 512], F32, name="psR")
                        nc.tensor.matmul(psR[:isz, :], h2b[:, i0:i0 + isz],
                                         wrp_tile[:, n0:n0 + nsz],
                                         start=True, stop=True)
                        st = stA if n0 < CA else stB
                        o0 = n0 if n0 < CA else n0 - CA
                        if j % 2 == 0:
                            nc.vector.tensor_copy(
                                st[:isz, o0:o0 + nsz], psR[:isz, :])
                        else:
                            nc.scalar.copy(
                                st[:isz, o0:o0 + nsz], psR[:isz, :])
                    nc.scalar.dma_start(recon[t, i0:i0 + isz, 0:CA],
                                        stA[:isz, :])
                    nc.scalar.dma_start(recon[t, i0:i0 + isz, CA:NP],
                                        stB[:isz, :])

            pending = None   # (t, tiles, h1T_own, gout) awaiting L2+recon
            for t in range(T):
                tl = loads(t)
                h1T_own, gout = do_l1(t, tl)
                if pending is not None:
                    pt, ptl, ph1, pgout = pending
                    h2b = do_l2(pt, ptl, ph1, pgout)
                    emit_recon(pt, h2b, ptl["wrp"])
                pending = (t, tl, h1T_own, gout)
            pt, ptl, ph1, pgout = pending
            h2b = do_l2(pt, ptl, ph1, pgout)
            emit_recon(pt, h2b, ptl["wrp"])
    nc.compile()
    return nc


def _chunked(x):
    """[NP, H] row-major -> [128, KCH*H] where out[p, k*H+h] = x[k*128+p, h]."""
    return np.ascontiguousarray(
        x.reshape(KCH, 128, H).transpose(1, 0, 2).reshape(128, KCH * H))


def kernel(emb, edge_index, W1l, b1, W1r, W2l, b2, W2r, Wrec, brec):
    global LAST_EXEC_NS, LAST_NS_PARTS
    LAST_EXEC_NS = None
    LAST_NS_PARTS = None

    emb = np.asarray(emb, dtype=np.float32)
    ei = np.asarray(edge_index)
    W1l, b1, W1r = (np.asarray(a, np.float32) for a in (W1l, b1, W1r))
    W2l, b2, W2r = (np.asarray(a, np.float32) for a in (W2l, b2, W2r))
    Wrec, brec = np.asarray(Wrec, np.float32), np.asarray(brec, np.float32)

    # ---- host prep: edge-count matrices (fp8 chunk-packed), degrees,
    # padded/transposed weight views
    ctc = np.zeros((NC, T, 128, KCH * S), dtype=ml_dtypes.float8_e4m3)
    rdeg = np.zeros((T, NP), dtype=np.float32)
    for t in range(T):
        src = ei[t, 0].astype(np.int64)
        dst = ei[t, 1].astype(np.int64)
        cnt = np.bincount(src * NP + dst, minlength=N * NP).reshape(N, NP)
        deg = np.bincount(dst, minlength=N)
        rdeg[t, :N] = 1.0 / np.maximum(deg, 1.0)
        cnt = cnt.astype(np.float32)
        for c in range(NC):
            blk = np.zeros((NP, S), np.float32)
            blk[:N] = cnt[:, c * S:(c + 1) * S]
            ctc[c, t] = np.ascontiguousarray(
                blk.reshape(KCH, 128, S).transpose(1, 0, 2)
                .reshape(128, KCH * S)).astype(ml_dtypes.float8_e4m3)

    embp = np.zeros((NP, H), dtype=np.float32)
    embp[:N] = emb
    xch_bf = _chunked(embp).astype(BF16)
    embT = np.ascontiguousarray(embp.T)                      # [64, NP] f32
    rb = np.ascontiguousarray(
        np.broadcast_to(rdeg[:, None, :], (T, H, NP)))       # [T, 64, NP]
    w1lT = np.ascontiguousarray(W1l.transpose(0, 2, 1))
    w1rT = np.ascontiguousarray(W1r.transpose(0, 2, 1))
    w2lT = np.ascontiguousarray(W2l.transpose(0, 2, 1))
    w2rT = np.ascontiguousarray(W2r.transpose(0, 2, 1))
    b1c = np.ascontiguousarray(b1[:, :, None])
    b2c = np.ascontiguousarray(b2[:, :, None])
    wrp = np.zeros((T, H + 1, NP), dtype=BF16)
    for t in range(T):
        wrp[t, :H, :N] = Wrec[t].T
        wrp[t, H, :N] = brec[t]

    core_ids = list(range(NC))
    trace = bool(os.environ.get("BASS_TRACE"))
    if trace:
        _ensure_ntff_hook()

    ncm = _build_merged()
    in_maps = [
        dict(ctc=ctc[c], xch=xch_bf,
             xT=np.ascontiguousarray(embT[:, c * S:(c + 1) * S]),
             rb=np.ascontiguousarray(rb[:, :, c * S:(c + 1) * S]),
             w1l=w1lT, w1r=w1rT, b1=b1c,
             w2l=w2lT, w2r=w2rT, b2=b2c, wrp=wrp)
        for c in core_ids
    ]
    rm = run_bass_kernel_spmd(ncm, in_maps, core_ids, trace=trace)

    h2T = np.concatenate([rm.results[c]["h2T"] for c in core_ids], axis=2)
    embeddings = np.ascontiguousarray(h2T.transpose(0, 2, 1)[:, :N, :])
    reconstructions = np.empty((T, N, N), dtype=np.float32)
    for c in core_ids:
        r0 = c * S
        r1 = min(r0 + S, N)
        if r1 > r0:
            reconstructions[:, r0:r1, :] = \
                rm.results[c]["recon"][:, :r1 - r0, :N]

    if rm.exec_time_ns is not None:
        LAST_EXEC_NS = int(rm.exec_time_ns)
        LAST_NS_PARTS = (rm.exec_time_ns,)
    return embeddings, reconstructions, emb


# revision 36
# speedup vs baseline: 1.0317x; 1.0317x over previous
"""Trainium2 Bass kernel for DangoPreTrain-style GNN pretraining forward.

Model (per edge type t of 3):
    h1 = relu(SAGE(emb, edges_t; W1l, b1, W1r))
    h2 = relu(SAGE(h1,  edges_t; W2l, b2, W2r))
    recon = h2 @ Wrec_t.T + brec_t            # [N, N]
Returns (embeddings [3,N,64], reconstructions [3,N,N], x_init [N,64]).

Strategy: the sparse mean-aggregation agg[i] = mean_{(j->i)} x[j] is
reformulated as a dense GEMM with the transposed edge-count matrix
CT[src, dst] = #edges(src->dst) (counts <= ~4, exact in fp8_e4m3):
aggT = x^T @ CT, then scale columns by 1/deg. CT is built on host
(bincount), chunk-packed in fp8_e4m3, and sharded over the 8 NeuronCores
by destination-node blocks (832 columns each). One SPMD launch does
everything, software-pipelined one edge type deep: each core loads its
CT shard once into SBUF and reuses it for both SAGE layers; h1 is
exchanged between cores with an on-device AllGather (layer-1 output is
PE-transposed to row layout first); the AllGather of type t overlaps
layer-1 of type t+1 and the recon slab of type t-1. Aggregation GEMMs
run bf16 x fp8 with fp32 PSUM accumulation; the 64x64 linear transforms
run in float32r; recon drains PSUM->SBUF on DVE+ACT and streams
contiguous row-stripes to HBM.
"""

import os
import sys

import numpy as np
import ml_dtypes

sys.path.insert(0, "/opt/trn_rl_repo")

import concourse.bacc as bacc
import concourse.mybir as mybir
import concourse.tile as tile
from concourse.bass_utils import run_bass_kernel_spmd


def _ensure_ntff_hook():
    """Register the NTFF profile hook missing from this image's antenv.

    trn_boot tries to install it but antenv lacks the axon_hooks module;
    shim it in-process so run_bass_kernel_spmd(trace=True) can profile.
    """
    try:
        from antenv import axon_hooks  # noqa: F401
        return True
    except ImportError:
        pass
    try:
        import types
        import antenv
        if "/root/.axon_site" not in sys.path:
            sys.path.insert(0, "/root/.axon_site")
        from trn_agent_boot.trn_boot import _ntff_profile_via_ctypes
        hook = _ntff_profile_via_ctypes("/opt/axon/libaxon_pjrt.so")
        if hook is None:
            return False
        m = types.ModuleType("antenv.axon_hooks")
        m.get_axon_ntff_profile_hook = lambda: hook
        m.set_axon_ntff_profile_hook = lambda h: None
        sys.modules["antenv.axon_hooks"] = m
        antenv.axon_hooks = m
        return True
    except Exception:
        return False


BF16 = ml_dtypes.bfloat16
F32 = mybir.dt.float32
BF = mybir.dt.bfloat16
RELU = mybir.ActivationFunctionType.Relu

N = 6607      # nodes
H = 64        # hidden
T = 3         # edge types
NP = 6656     # N padded to 52*128
KCH = NP // 128   # 52 contraction chunks
NC = 8        # cores
S = NP // NC  # 832 dst columns per core
FB = [(0, 512), (512, 320)]          # free-dim blocks of the 832-wide shard
IB = [(i * 128, 128) for i in range(6)] + [(768, 64)]  # 832-row blocks
NB = [(i * 512, 512) for i in range(13)]               # recon col blocks

LAST_EXEC_NS = None
LAST_NS_PARTS = None


def _build_merged():
    """Single-launch kernel, software-pipelined one type deep:
    L1(t) -> AllGather(t) overlaps L1(t+1); L2(t)/recon(t) run after.
    CT shard loaded once per type (fp8_e4m3, counts exact), reused by
    both layers from SBUF."""
    from concourse.masks import make_identity
    FP8 = mybir.dt.float8e4
    F32R = mybir.dt.float32r

    nc = bacc.Bacc("TRN2", target_bir_lowering=False, debug=False,
                   num_devices=NC)
    ctc = nc.dram_tensor("ctc", [T, 128, KCH * S], FP8, kind="ExternalInput")
    xch = nc.dram_tensor("xch", [128, KCH * H], BF, kind="ExternalInput")
    xT = nc.dram_tensor("xT", [H, S], F32R, kind="ExternalInput")
    rb = nc.dram_tensor("rb", [T, H, S], F32, kind="ExternalInput")
    w1l = nc.dram_tensor("w1l", [T, H, H], F32R, kind="ExternalInput")
    w1r = nc.dram_tensor("w1r", [T, H, H], F32R, kind="ExternalInput")
    b1 = nc.dram_tensor("b1", [T, H, 1], F32, kind="ExternalInput")
    w2l = nc.dram_tensor("w2l", [T, H, H], F32R, kind="ExternalInput")
    w2r = nc.dram_tensor("w2r", [T, H, H], F32R, kind="ExternalInput")
    b2 = nc.dram_tensor("b2", [T, H, 1], F32, kind="ExternalInput")
    wrp = nc.dram_tensor("wrp", [T, H + 1, NP], BF, kind="ExternalInput")
    h2T = nc.dram_tensor("h2T", [T, H, S], F32, kind="ExternalOutput")
    recon = nc.dram_tensor("recon", [T, S, NP], F32, kind="ExternalOutput")

    CA = 7 * 512
    CB = NP - CA

    with tile.TileContext(nc) as tc:
        with (
            tc.tile_pool(name="const", bufs=1) as const,
            tc.tile_pool(name="ctp", bufs=2) as ctp,
            tc.tile_pool(name="hin", bufs=1) as hin,
            tc.tile_pool(name="wrpp", bufs=2) as wrpp,
            tc.tile_pool(name="wt", bufs=2) as wt,
            tc.tile_pool(name="work", bufs=2) as work,
            tc.tile_pool(name="outp", bufs=2) as outp,
            tc.tile_pool(name="psum_a", bufs=1, space="PSUM") as psum_a,
            tc.tile_pool(name="psum_w", bufs=1, space="PSUM") as psum_w,
            tc.tile_pool(name="psum_t", bufs=1, space="PSUM") as psum_t,
            tc.tile_pool(name="psum_r", bufs=4, space="PSUM") as psum_r,
            tc.tile_pool(name="dram", bufs=3, space="DRAM") as dram,
        ):
            xch_tile = const.tile([128, KCH * H], BF)
            nc.sync.dma_start(xch_tile[:], xch[:])
            xT_tile = const.tile([H, S], F32R)
            nc.sync.dma_start(xT_tile[:], xT[:])
            ident = const.tile([H, H], BF)
            make_identity(nc, ident)

            def loads(t):
                ct_big = ctp.tile([128, KCH * S], FP8, tag="ct",
                                  name="ct_big")
                nc.sync.dma_start(ct_big[:], ctc[t])
                wrp_tile = wrpp.tile([H + 1, NP], BF, tag="wrp",
                                     name="wrp_tile")
                nc.sync.dma_start(wrp_tile[:], wrp[t])
                tl = dict(ct=ct_big, wrp=wrp_tile)
                for nm, src_ in (("rb", rb), ("w1l", w1l), ("w1r", w1r),
                                 ("w2l", w2l), ("w2r", w2r)):
                    dt_ = F32 if nm == "rb" else F32R
                    shp = [H, S] if nm == "rb" else [H, H]
                    tile_ = wt.tile(shp, dt_, tag=nm, name=nm + "_t")
                    nc.sync.dma_start(tile_[:], src_[t])
                    tl[nm] = tile_
                for nm, src_ in (("b1", b1), ("b2", b2)):
                    tile_ = wt.tile([H, 1], F32, tag=nm, name=nm + "_t")
                    nc.sync.dma_start(tile_[:], src_[t])
                    tl[nm] = tile_
                return tl

            def agg(lhs_tile, ct_big, rb_tile, wl, wr, xTr, bcol, out):
                psA = psum_a.tile([64, 512], F32, tag="psA", name="psA")
                psB = psum_a.tile([64, 320], F32, tag="psB", name="psB")
                for k in range(KCH):
                    lhs = lhs_tile[:, k * H:(k + 1) * H]
                    nc.tensor.matmul(psA[:], lhs,
                                     ct_big[:, k * S:k * S + 512],
                                     start=(k == 0), stop=(k == KCH - 1))
                    nc.tensor.matmul(psB[:], lhs,
                                     ct_big[:, k * S + 512:(k + 1) * S],
                                     start=(k == 0), stop=(k == KCH - 1))
                mm = work.tile([64, S], mybir.dt.float32r, tag="mm",
                               name="mm", bufs=1)
                nc.vector.tensor_mul(mm[:, 0:512], psA[:], rb_tile[:, 0:512])
                nc.vector.tensor_mul(mm[:, 512:S], psB[:], rb_tile[:, 512:S])
                for (f0, fs) in FB:
                    psW = psum_w.tile([64, 512], F32, name="psW")
                    nc.tensor.matmul(psW[:, :fs], wl[:], mm[:, f0:f0 + fs],
                                     start=True, stop=False)
                    nc.tensor.matmul(psW[:, :fs], wr[:], xTr[:, f0:f0 + fs],
                                     start=False, stop=True)
                    nc.scalar.activation(out[:, f0:f0 + fs], psW[:, :fs],
                                         RELU, bias=bcol[:])

            def do_l1(t, tl):
                h1T_own = work.tile([H, S], mybir.dt.float32r,
                                    tag="h1T_own", name="h1T_own")
                agg(xch_tile, tl["ct"], tl["rb"], tl["w1l"], tl["w1r"],
                    xT_tile, tl["b1"], h1T_own)
                h1Tb = work.tile([H, S], BF, tag="h1Tb", name="h1Tb",
                                 bufs=1)
                nc.vector.tensor_copy(h1Tb[:], h1T_own[:])
                gin = dram.tile([S, H], BF, tag="gin", name="gin")
                gout = dram.tile([NP, H], BF, tag="gout", name="gout",
                                 addr_space="Shared")
                for (j0, jw) in IB:
                    psT = psum_t.tile([128, H], BF, name="psT")
                    nc.tensor.transpose(psT[:jw, :], h1Tb[:, j0:j0 + jw],
                                        ident[:])
                    trs = work.tile([128, H], BF, tag="trs", name="trs")
                    nc.vector.tensor_copy(trs[:jw, :], psT[:jw, :])
                    nc.scalar.dma_start(gin[j0:j0 + jw, :], trs[:jw, :])
                nc.gpsimd.collective_compute(
                    "AllGather", mybir.AluOpType.bypass,
                    replica_groups=[list(range(NC))],
                    ins=[gin[:]], outs=[gout[:]],
                )
                return h1T_own, gout

            def do_l2(t, tl, h1T_own, gout):
                h1ch = hin.tile([128, KCH * H], BF, name="h1ch")
                for k in range(KCH):
                    nc.scalar.dma_start(h1ch[:, k * H:(k + 1) * H],
                                        gout[k * 128:(k + 1) * 128, :])
                hf = outp.tile([H, S], F32, tag="hf", name="hf", bufs=1)
                agg(h1ch, tl["ct"], tl["rb"], tl["w2l"], tl["w2r"],
                    h1T_own, tl["b2"], hf)
                nc.sync.dma_start(h2T[t], hf[:])
                h2b = work.tile([H + 1, S], BF, tag="h2b", name="h2b",
                                bufs=1)
                nc.vector.tensor_copy(h2b[0:H, :], hf[:])
                nc.vector.memset(h2b[H:H + 1, :], 1.0)
                return h2b

            def emit_recon(t, h2b, wrp_tile):
                for (i0, isz) in IB:
                    stA = outp.tile([128, CA], F32, tag="stA", name="stA")
                    stB = outp.tile([128, CB], F32, tag="stB", name="stB")
                    for j, (n0, nsz) in enumerate(NB):
                        psR = psum_r.tile([128, 512], F32, name="psR")
                        nc.tensor.matmul(psR[:isz, :], h2b[:, i0:i0 + isz],
                                         wrp_tile[:, n0:n0 + nsz],
                                         start=True, stop=True)
                        st = stA if n0 < CA else stB
                        o0 = n0 if n0 < CA else n0 - CA
                        if j % 2 == 0:
                            nc.vector.tensor_copy(
                                st[:isz, o0:o0 + nsz], psR[:isz, :])
                        else:
                            nc.scalar.copy(
                                st[:isz, o0:o0 + nsz], psR[:isz, :])
                    nc.sync.dma_start(recon[t, i0:i0 + isz, 0:CA],
                                      stA[:isz, :])
                    nc.sync.dma_start(recon[t, i0:i0 + isz, CA:NP],
                                      stB[:isz, :])

            pending = None   # (t, tiles, h1T_own, gout) awaiting L2+recon
            for t in range(T):
                tl = loads(t)
                h1T_own, gout = do_l1(t, tl)
                if pending is not None:
                    pt, ptl, ph1, pgout = pending
                    h2b = do_l2(pt, ptl, ph1, pgout)
                    emit_recon(pt, h2b, ptl["wrp"])
                pending = (t, tl, h1T_own, gout)
            pt, ptl, ph1, pgout = pending
            h2b = do_l2(pt, ptl, ph1, pgout)
            emit_recon(pt, h2b, ptl["wrp"])
    nc.compile()
    return nc


def _chunked(x):
    """[NP, H] row-major -> [128, KCH*H] where out[p, k*H+h] = x[k*128+p, h]."""
    return np.ascontiguousarray(
        x.reshape(KCH, 128, H).transpose(1, 0, 2).reshape(128, KCH * H))


def kernel(emb, edge_index, W1l, b1, W1r, W2l, b2, W2r, Wrec, brec):
    global LAST_EXEC_NS, LAST_NS_PARTS
    LAST_EXEC_NS = None
    LAST_NS_PARTS = None

    emb = np.asarray(emb, dtype=np.float32)
    ei = np.asarray(edge_index)
    W1l, b1, W1r = (np.asarray(a, np.float32) for a in (W1l, b1, W1r))
    W2l, b2, W2r = (np.asarray(a, np.float32) for a in (W2l, b2, W2r))
    Wrec, brec = np.asarray(Wrec, np.float32), np.asarray(brec, np.float32)

    # ---- host prep: edge-count matrices (fp8 chunk-packed), degrees,
    # padded/transposed weight views
    ctc = np.zeros((NC, T, 128, KCH * S), dtype=ml_dtypes.float8_e4m3)
    rdeg = np.zeros((T, NP), dtype=np.float32)
    for t in range(T):
        src = ei[t, 0].astype(np.int64)
        dst = ei[t, 1].astype(np.int64)
        cnt = np.bincount(src * NP + dst, minlength=N * NP).reshape(N, NP)
        deg = np.bincount(dst, minlength=N)
        rdeg[t, :N] = 1.0 / np.maximum(deg, 1.0)
        cnt = cnt.astype(np.float32)
        for c in range(NC):
            blk = np.zeros((NP, S), np.float32)
            blk[:N] = cnt[:, c * S:(c + 1) * S]
            ctc[c, t] = np.ascontiguousarray(
                blk.reshape(KCH, 128, S).transpose(1, 0, 2)
                .reshape(128, KCH * S)).astype(ml_dtypes.float8_e4m3)

    embp = np.zeros((NP, H), dtype=np.float32)
    embp[:N] = emb
    xch_bf = _chunked(embp).astype(BF16)
    embT = np.ascontiguousarray(embp.T)                      # [64, NP] f32
    rb = np.ascontiguousarray(
        np.broadcast_to(rdeg[:, None, :], (T, H, NP)))       # [T, 64, NP]
    w1lT = np.ascontiguousarray(W1l.transpose(0, 2, 1))
    w1rT = np.ascontiguousarray(W1r.transpose(0, 2, 1))
    w2lT = np.ascontiguousarray(W2l.transpose(0, 2, 1))
    w2rT = np.ascontiguousarray(W2r.transpose(0, 2, 1))
    b1c = np.ascontiguousarray(b1[:, :, None])
    b2c = np.ascontiguousarray(b2[:, :, None])
    wrp = np.zeros((T, H + 1, NP), dtype=BF16)
    for t in range(T):
        wrp[t, :H, :N] = Wrec[t].T
        wrp[t, H, :N] = brec[t]

    core_ids = list(range(NC))
    trace = bool(os.environ.get("BASS_TRACE"))
    if trace:
        _ensure_ntff_hook()

    ncm = _build_merged()
    in_maps = [
        dict(ctc=ctc[c], xch=xch_bf,
             xT=np.ascontiguousarray(embT[:, c * S:(c + 1) * S]),
             rb=np.ascontiguousarray(rb[:, :, c * S:(c + 1) * S]),
             w1l=w1lT, w1r=w1rT, b1=b1c,
             w2l=w2lT, w2r=w2rT, b2=b2c, wrp=wrp)
        for c in core_ids
    ]
    rm = run_bass_kernel_spmd(ncm, in_maps, core_ids, trace=trace)

    h2T = np.concatenate([rm.results[c]["h2T"] for c in core_ids], axis=2)
    embeddings = np.ascontiguousarray(h2T.transpose(0, 2, 1)[:, :N, :])
    reconstructions = np.empty((T, N, N), dtype=np.float32)
    for c in core_ids:
        r0 = c * S
        r1 = min(r0 + S, N)
        if r1 > r0:
            reconstructions[:, r0:r1, :] = \
                rm.results[c]["recon"][:, :r1 - r0, :N]

    if rm.exec_time_ns is not None:
        LAST_EXEC_NS = int(rm.exec_time_ns)
        LAST_NS_PARTS = (rm.exec_time_ns,)
    return embeddings, reconstructions, emb
